# revision 20
# baseline (speedup 1.0000x reference)
"""DeTPP loss kernel for 8 TRN2 NeuronCores (batch-parallel SPMD Bass/Tile).

Strategy: shard along batch B (8 per core). Host prep does index plumbing
on tiny tensors (row ids, the assignment cost matrix built from the K*T
picked-category logits and L1 terms: ~0.3MB/core vs the 32MB logits
table). The memory-heavy work stays on device, per core:
  - one bf16 logits table [R, K*C] in DRAM; the unique needed rows
    (~1840 of 8192, deduped, sorted by address for DRAM locality; 2KB
    each) are fetched with 2-tile chunked indirect row-gather DMAs (one
    SWDGE descriptor per row, 16 HW DMA engines in parallel),
  - 24-permutation assignment totals via PE (transpose + block-diagonal
    0/1 matmul per half) and a DVE segmented min = exact Hungarian
    optimum for K=4 -> per-item min cost, DMA'd out mid-stream,
  - ACT exp in wide instructions pipelined behind the gather chunks;
    per-(row,k) softmax denominators collapsed 256->16 by a tree of
    2x-mode bf16 tensor_tensor adds on DVE (tensor_reduce has no DVE
    fast mode), partial sums DMA'd out per group as they complete.
Host finishes the scalar epilogue on the shipped partials (16-way sum,
ln, map rows->items, masked mean) - ~1M flops vs the device's ~6M heavy
ops on 4MB. The row-slot count is data-dependent; the Bass program is
compiled lazily per slot-tile count (15 tiles for typical inputs).
"""
import sys

sys.path.insert(0, '/opt/trn_rl_repo')

import itertools
import numpy as np
import ml_dtypes

BF16 = ml_dtypes.bfloat16

L, B, I, K, C = 1024, 64, 256, 4, 256
BS = B // 8            # batch per core
R = L * BS             # rows per core (8192), row id r = l*BS + b
N = I * BS             # items per core (2048)
NT = N // 128          # 16 item-tiles; item n sits at (n // NT, n % NT)
PERMS = np.array(list(itertools.permutations(range(K))), dtype=np.int32)
NP_ = PERMS.shape[0]   # 24

# bigc packed-constant column layout (all bf16)
O_COST, O_PMAT, O_ID = 0, 256, 448
W_BIGC = 576


NDENSE = 4             # leading slot-tiles delivered dense (host-gathered)


def _chunks(ntu):
    nd = min(NDENSE, ntu)
    ch = [(t, 2) for t in range(nd, ntu - 1, 2)]
    if (ntu - nd) % 2:
        ch.append((ntu - 1, 1))
    return ch


def _dense_parts(ntu):
    nd = min(NDENSE, ntu)
    return [(0, 1), (1, 1), (2, 2)] if nd == 4 else [(0, nd)]


def _exps(ntu):
    return _dense_parts(ntu) + _chunks(ntu)


def _groups(ntu):
    gs, t = [], 0
    while ntu - t > 4:
        gs.append((t, 4))
        t += 4
    if ntu - t == 4:
        gs.append((t, 2))
        gs.append((t + 2, 2))
    elif ntu - t == 3:
        gs.append((t, 2))
        gs.append((t + 2, 1))
    else:
        gs.append((t, ntu - t))
    return gs


def _slotmap(ntu):
    # slot (p, t) <-> sorted-row rank so each gather chunk's descriptors
    # hit ascending table rows; chunk (st, w) enumerates its offset AP
    # p-major: rank = st*128 + p*w + (t - st).
    sm = np.empty((128, ntu), np.int64)
    for st, w in _dense_parts(ntu) + _chunks(ntu):
        for p in range(128):
            sm[p, st:st + w] = st * 128 + p * w + np.arange(w)
    return sm


def _host_prep(core, time, amount, out_time, out_amount, out_cat_logits, cat,
               lengths, indices, consts):
    """Phase 1: per-core item math; returns pieces + unique sorted rows."""
    bsl = slice(core * BS, (core + 1) * BS)
    idx = indices[:, bsl].astype(np.int64)                    # (I, BS)
    bb = np.broadcast_to(np.arange(BS)[None, :], idx.shape)   # (I, BS)
    pos = (idx[:, :, None] + 1 + np.arange(K)[None, None, :]) % L  # (I,BS,K)
    bb3 = np.broadcast_to(bb[:, :, None], pos.shape)

    tloc = time[:, bsl]
    dt = tloc[pos, bb3] - tloc[idx, bb][:, :, None]           # (I, BS, K)
    aw = amount[:, bsl][pos, bb3]                             # (I, BS, K)
    cw = cat[:, bsl][pos, bb3].astype(np.int64)               # (I, BS, K)
    ot = out_time[:, bsl][idx, bb]                            # (I, BS, K)
    oa = out_amount[:, bsl][idx, bb]                          # (I, BS, K)
    ocl = out_cat_logits[:, bsl]                              # (L, BS, K, C)
    kk = np.arange(K)[None, None, :, None]
    picked = ocl[idx[:, :, None, None], bb[:, :, None, None], kk,
                 cw[:, :, None, :]]                           # (I, BS, K, T)
    # cost[n, k, t] = |ot_k - dt_t| + |oa_k - a_t| - picked_logit[k, t];
    # the lse part of the CE is added after the assignment min (it is
    # constant across the permutation search).
    cost = (np.abs(ot[:, :, :, None] - dt[:, :, None, :])
            + np.abs(oa[:, :, :, None] - aw[:, :, None, :])
            - picked)                                          # (I, BS, K, T)
    valid = (idx + K < lengths[bsl].astype(np.int64)[None, :])

    bigc = np.zeros((128, W_BIGC), BF16)
    bigc[:, O_COST:O_COST + 256] = cost.reshape(128, 256).astype(BF16)
    bigc[:, O_PMAT:O_PMAT + 192] = consts["pmat"]
    bigc[:, O_ID:O_ID + 128] = consts["ident"]

    rows = (idx * BS + bb).reshape(N)
    uniq = np.unique(rows)                                    # sorted
    aug = np.ascontiguousarray(out_cat_logits[:, bsl]).reshape(R, K * C)
    return {"aug": aug.astype(BF16), "bigc": bigc, "rows": rows,
            "uniq": uniq, "valid": valid.reshape(N)}


def _finish_prep(prep, ntu, slotmap):
    """Phase 2: place unique rows at slots, build item->slot maps."""
    uniq = prep["uniq"]
    upad = np.concatenate([uniq, np.full(ntu * 128 - len(uniq), uniq[0],
                                         np.int64)])
    rowidx = upad[slotmap].astype(np.int32)                   # (128, ntu)
    rank_of_item = np.searchsorted(uniq, prep["rows"])        # (N,)
    # inverse of slotmap: rank -> (p, t)
    inv_p = np.empty(ntu * 128, np.int64)
    inv_t = np.empty(ntu * 128, np.int64)
    pp, tt = np.indices(slotmap.shape)
    inv_p[slotmap.reshape(-1)] = pp.reshape(-1)
    inv_t[slotmap.reshape(-1)] = tt.reshape(-1)
    nd = min(NDENSE, ntu)
    gdense = prep["aug"][rowidx[:, :nd].reshape(-1)].reshape(128, -1)
    in_map = {"aug": prep["aug"], "bigc": prep["bigc"], "rowidx": rowidx,
              "gdense": gdense}
    return in_map, (inv_p[rank_of_item], inv_t[rank_of_item], prep["valid"])


def _make_consts():
    pmat1 = np.zeros((K * K, NP_), np.float32)
    for p in range(NP_):
        for k in range(K):
            pmat1[k * K + PERMS[p, k], p] = 1.0
    pmat = np.zeros((128, 8 * NP_), np.float32)
    for tblk in range(8):
        pmat[tblk * 16:(tblk + 1) * 16, tblk * NP_:(tblk + 1) * NP_] = pmat1
    return {"pmat": pmat.astype(BF16), "ident": np.eye(128, dtype=BF16)}


def _build(nc, bass, mybir, tile, ntu):
    AP = bass.AP
    dt = mybir.dt
    Alu = mybir.AluOpType
    Act = mybir.ActivationFunctionType

    nd = min(NDENSE, ntu)
    aug = nc.dram_tensor("aug", [R, K * C], dt.bfloat16, kind="ExternalInput")
    bigc = nc.dram_tensor("bigc", [128, W_BIGC], dt.bfloat16,
                          kind="ExternalInput")
    rowidx = nc.dram_tensor("rowidx", [128, ntu], dt.int32,
                            kind="ExternalInput")
    gdense = nc.dram_tensor("gdense", [128, nd * K * C], dt.bfloat16,
                            kind="ExternalInput")
    mint_o = nc.dram_tensor("mint_o", [128, NT], dt.float32,
                            kind="ExternalOutput")
    s16_o = nc.dram_tensor("s16_o", [128, ntu * K * 16], dt.bfloat16,
                           kind="ExternalOutput")

    with tile.TileContext(nc) as tc:
        with (
            tc.tile_pool(name="main", bufs=1) as pool,
            tc.tile_pool(name="psum", bufs=1, space="PSUM") as ppool,
        ):
            # dense-primed leading tiles land on fast HW queues while the
            # SWDGE warms up; finely split so exp starts ASAP. First dense
            # part goes ahead of everything (it gates exp0); rowidx second
            # (it gates the indirect chain, which has slack).
            G = pool.tile([128, ntu * K * C], dt.bfloat16)
            ri = pool.tile([128, ntu], dt.int32)
            parts = _dense_parts(ntu)
            nc.sync.dma_start(G[:, :parts[0][1] * 1024],
                              gdense.ap()[:, :parts[0][1] * 1024])
            nc.sync.dma_start(ri[:], rowidx.ap())
            for st, w in parts[1:]:
                nc.sync.dma_start(
                    G[:, st * 1024:(st + w) * 1024],
                    gdense.ap()[:, st * 1024:(st + w) * 1024])
            cb = pool.tile([128, W_BIGC], dt.bfloat16)
            nc.sync.dma_start(cb[:], bigc.ap())

            pmat_v = cb[:, O_PMAT:O_PMAT + 192]
            ident_v = cb[:, O_ID:O_ID + 128]

            # dummy 16-row gather warms the SW-DMA engines before rowidx
            # lands (offsets are memset, not DMA-dependent)
            dum_i = pool.tile([16, 1], dt.int32)
            nc.gpsimd.memset(dum_i[:], 0)
            dum_o = pool.tile([16, K * C], dt.bfloat16)
            nc.gpsimd.indirect_dma_start(
                out=dum_o[:], out_offset=None, in_=aug.ap(),
                in_offset=bass.IndirectOffsetOnAxis(ap=dum_i[:], axis=0))

            # ---- indirect row gathers, chunked for DMA/compute overlap
            for st, w in _chunks(ntu):
                nc.gpsimd.indirect_dma_start(
                    out=G[:, st * 1024:(st + w) * 1024], out_offset=None,
                    in_=aug.ap(),
                    in_offset=bass.IndirectOffsetOnAxis(ap=ri[:, st:st + w],
                                                        axis=0))

            # ---- 24-perm totals per half on PE, segmented min on DVE
            # (runs while the gathers stream; cost matrix is host-built)
            mint = pool.tile([128, NT], dt.float32)
            for h in range(2):
                pT = ppool.tile([128, 128], dt.bfloat16, tag=f"pT{h}")
                nc.tensor.transpose(out=pT[:],
                                    in_=cb[:, O_COST + h * 128:
                                           O_COST + (h + 1) * 128],
                                    identity=ident_v)
                cT = pool.tile([128, 128], dt.bfloat16, tag=f"cT{h}")
                nc.vector.tensor_copy(out=cT[:], in_=pT[:])
                ptot = ppool.tile([128, 8 * NP_], dt.float32, tag=f"ptot{h}")
                nc.tensor.matmul(out=ptot[:], lhsT=cT[:], rhs=pmat_v,
                                 start=True, stop=True)
                nc.vector.tensor_reduce(
                    out=mint[:, h * 8:(h + 1) * 8],
                    in_=ptot[:].rearrange("p (t q) -> p t q", q=NP_),
                    axis=mybir.AxisListType.X, op=Alu.min)
            nc.sync.dma_start(mint_o.ap(), mint[:])

            # ---- exp (bf16), pipelined behind the gather chunks
            E = pool.tile([128, ntu * K * C], dt.bfloat16)
            for st, w in _exps(ntu):
                nc.scalar.activation(out=E[:, st * 1024:(st + w) * 1024],
                                     in_=G[:, st * 1024:(st + w) * 1024],
                                     func=Act.Exp)

            # ---- softmax denominators: per group, a tree of 2x-mode bf16
            # adds collapses (segs x 256) -> (segs x 16); host sums the 16.
            s16 = pool.tile([128, ntu * K * 16], dt.bfloat16)
            tree = []
            for w in (128, 64, 32):
                trtile = pool.tile([128, 4 * K * w], dt.bfloat16,
                                   tag=f"tr{w}", name=f"tr{w}")
                tree.append(trtile)

            def ev(nseg, segstep, width, off):
                a = E[:, 0:1]
                return AP(a.tensor, a.offset + off,
                          [list(a.ap[0]), [segstep, nseg], [1, width]])

            def tv(tl, nseg, segstride, width, off):
                a = tl[:, 0:1]
                return AP(a.tensor, a.offset + off,
                          [list(a.ap[0]), [segstride, nseg], [1, width]])

            def sv(nseg, width, off):
                a = s16[:, 0:1]
                return AP(a.tensor, a.offset + off,
                          [list(a.ap[0]), [16, nseg], [1, width]])

            for gt, gw in _groups(ntu):
                ns = gw * K
                nc.vector.tensor_tensor(
                    out=tv(tree[0], ns, 128, 128, 0),
                    in0=ev(ns, 256, 128, gt * 1024),
                    in1=ev(ns, 256, 128, gt * 1024 + 128), op=Alu.add)
                nc.vector.tensor_tensor(
                    out=tv(tree[1], ns, 64, 64, 0),
                    in0=tv(tree[0], ns, 128, 64, 0),
                    in1=tv(tree[0], ns, 128, 64, 64), op=Alu.add)
                nc.vector.tensor_tensor(
                    out=tv(tree[2], ns, 32, 32, 0),
                    in0=tv(tree[1], ns, 64, 32, 0),
                    in1=tv(tree[1], ns, 64, 32, 32), op=Alu.add)
                nc.vector.tensor_tensor(
                    out=sv(ns, 16, gt * K * 16),
                    in0=tv(tree[2], ns, 32, 16, 0),
                    in1=tv(tree[2], ns, 32, 16, 16), op=Alu.add)
                nc.sync.dma_start(
                    s16_o.ap()[:, gt * K * 16:(gt + gw) * K * 16],
                    s16[:, gt * K * 16:(gt + gw) * K * 16])
    return nc


NCORES = 8
_COMPILED = {}


def _get_compiled(ntu):
    if ntu not in _COMPILED:
        import concourse.bacc as bacc
        import concourse.bass as bass
        import concourse.mybir as mybir
        import concourse.tile as tile
        nc = bacc.Bacc("TRN2", target_bir_lowering=False, debug=False,
                       num_devices=NCORES)
        _build(nc, bass, mybir, tile, ntu)
        nc.compile()
        _COMPILED[ntu] = nc
    return _COMPILED[ntu]


def kernel(time, amount, out_time, out_amount, out_cat_logits, cat, lengths,
           indices):
    from concourse.bass_utils import run_bass_kernel_spmd

    time = np.asarray(time, dtype=np.float32)
    amount = np.asarray(amount, dtype=np.float32)
    out_time = np.asarray(out_time, dtype=np.float32)
    out_amount = np.asarray(out_amount, dtype=np.float32)
    out_cat_logits = np.asarray(out_cat_logits, dtype=np.float32)
    cat = np.asarray(cat, dtype=np.int32)
    lengths = np.asarray(lengths, dtype=np.int32)
    indices = np.asarray(indices, dtype=np.int32)

    consts = _make_consts()
    preps = [
        _host_prep(c, time, amount, out_time, out_amount, out_cat_logits,
                   cat, lengths, indices, consts)
        for c in range(NCORES)
    ]
    ntu = max((len(p["uniq"]) + 127) // 128 for p in preps)
    slotmap = _slotmap(ntu)
    nc = _get_compiled(ntu)
    in_maps, maps = [], []
    for p in preps:
        m, mp = _finish_prep(p, ntu, slotmap)
        in_maps.append(m)
        maps.append(mp)
    res = run_bass_kernel_spmd(nc, in_maps, core_ids=list(range(NCORES)))
    ls, cn = 0.0, 0.0
    for c in range(NCORES):
        mint = res.results[c]["mint_o"].reshape(N)            # item n-major
        s16 = res.results[c]["s16_o"].astype(np.float32)
        s4 = s16.reshape(128, ntu * K, 16).sum(-1)            # (128, ntu*K)
        slse = np.log(s4).reshape(128, ntu, K).sum(-1)        # (128, ntu)
        ip, it, valid = maps[c]
        item = mint + slse[ip, it]
        ls += float((item * valid).sum())
        cn += float(valid.sum())
    return np.float32(ls / (cn * K))


# revision 21
# speedup vs baseline: 1.1637x; 1.1637x over previous
"""DeTPP loss kernel for 8 TRN2 NeuronCores (batch-parallel SPMD Bass/Tile).

Strategy: shard along batch B (8 per core). Host prep does index plumbing
on tiny tensors (row ids, the assignment cost matrix built from the K*T
picked-category logits and L1 terms: ~0.3MB/core vs the 32MB logits
table). The memory-heavy work stays on device, per core:
  - one bf16 logits table [R, K*C] in DRAM; the unique needed rows
    (~1840 of 8192, deduped, sorted by address for DRAM locality; 2KB
    each) are fetched with 2-tile chunked indirect row-gather DMAs (one
    SWDGE descriptor per row, 16 HW DMA engines in parallel),
  - 24-permutation assignment totals via PE (transpose + block-diagonal
    0/1 matmul per half) and a DVE segmented min = exact Hungarian
    optimum for K=4 -> per-item min cost, DMA'd out mid-stream,
  - ACT exp in wide instructions pipelined behind the gather chunks;
    per-(row,k) softmax denominators collapsed 256->16 by a tree of
    2x-mode bf16 tensor_tensor adds on DVE (tensor_reduce has no DVE
    fast mode), partial sums DMA'd out per group as they complete.
Host finishes the scalar epilogue on the shipped partials (16-way sum,
ln, map rows->items, masked mean) - ~1M flops vs the device's ~6M heavy
ops on 4MB. The row-slot count is data-dependent; the Bass program is
compiled lazily per slot-tile count (15 tiles for typical inputs).
"""
import sys

sys.path.insert(0, '/opt/trn_rl_repo')

import itertools
import numpy as np
import ml_dtypes

BF16 = ml_dtypes.bfloat16

L, B, I, K, C = 1024, 64, 256, 4, 256
BS = B // 8            # batch per core
R = L * BS             # rows per core (8192), row id r = l*BS + b
N = I * BS             # items per core (2048)
NT = N // 128          # 16 item-tiles; item n sits at (n // NT, n % NT)
PERMS = np.array(list(itertools.permutations(range(K))), dtype=np.int32)
NP_ = PERMS.shape[0]   # 24

# bigc packed-constant column layout (all bf16)
O_COST, O_PMAT, O_ID = 0, 256, 448
W_BIGC = 576


NDENSE = 4             # leading slot-tiles delivered dense (host-gathered)


def _chunks(ntu):
    nd = min(NDENSE, ntu)
    ch = [(t, 2) for t in range(nd, ntu - 1, 2)]
    if (ntu - nd) % 2:
        ch.append((ntu - 1, 1))
    return ch


def _dense_parts(ntu):
    nd = min(NDENSE, ntu)
    return [(0, 1), (1, 1), (2, 2)] if nd == 4 else [(0, nd)]


def _exps(ntu):
    return _dense_parts(ntu) + _chunks(ntu)


def _groups(ntu):
    gs, t = [], 0
    while ntu - t > 4:
        gs.append((t, 4))
        t += 4
    if ntu - t == 4:
        gs.append((t, 2))
        gs.append((t + 2, 2))
    elif ntu - t == 3:
        gs.append((t, 2))
        gs.append((t + 2, 1))
    else:
        gs.append((t, ntu - t))
    return gs


def _slotmap(ntu):
    # slot (p, t) <-> sorted-row rank so each gather chunk's descriptors
    # hit ascending table rows; chunk (st, w) enumerates its offset AP
    # p-major: rank = st*128 + p*w + (t - st).
    sm = np.empty((128, ntu), np.int64)
    for st, w in _dense_parts(ntu) + _chunks(ntu):
        for p in range(128):
            sm[p, st:st + w] = st * 128 + p * w + np.arange(w)
    return sm


def _host_prep(core, time, amount, out_time, out_amount, out_cat_logits, cat,
               lengths, indices, consts):
    """Phase 1: per-core item math; returns pieces + unique sorted rows."""
    bsl = slice(core * BS, (core + 1) * BS)
    idx = indices[:, bsl].astype(np.int64)                    # (I, BS)
    bb = np.broadcast_to(np.arange(BS)[None, :], idx.shape)   # (I, BS)
    pos = (idx[:, :, None] + 1 + np.arange(K)[None, None, :]) % L  # (I,BS,K)
    bb3 = np.broadcast_to(bb[:, :, None], pos.shape)

    tloc = time[:, bsl]
    dt = tloc[pos, bb3] - tloc[idx, bb][:, :, None]           # (I, BS, K)
    aw = amount[:, bsl][pos, bb3]                             # (I, BS, K)
    cw = cat[:, bsl][pos, bb3].astype(np.int64)               # (I, BS, K)
    ot = out_time[:, bsl][idx, bb]                            # (I, BS, K)
    oa = out_amount[:, bsl][idx, bb]                          # (I, BS, K)
    ocl = out_cat_logits[:, bsl]                              # (L, BS, K, C)
    kk = np.arange(K)[None, None, :, None]
    picked = ocl[idx[:, :, None, None], bb[:, :, None, None], kk,
                 cw[:, :, None, :]]                           # (I, BS, K, T)
    # cost[n, k, t] = |ot_k - dt_t| + |oa_k - a_t| - picked_logit[k, t];
    # the lse part of the CE is added after the assignment min (it is
    # constant across the permutation search).
    cost = (np.abs(ot[:, :, :, None] - dt[:, :, None, :])
            + np.abs(oa[:, :, :, None] - aw[:, :, None, :])
            - picked)                                          # (I, BS, K, T)
    valid = (idx + K < lengths[bsl].astype(np.int64)[None, :])

    bigc = np.zeros((128, W_BIGC), BF16)
    bigc[:, O_COST:O_COST + 256] = cost.reshape(128, 256).astype(BF16)
    bigc[:, O_PMAT:O_PMAT + 192] = consts["pmat"]
    bigc[:, O_ID:O_ID + 128] = consts["ident"]

    rows = (idx * BS + bb).reshape(N)
    uniq = np.unique(rows)                                    # sorted
    aug = np.ascontiguousarray(out_cat_logits[:, bsl]).reshape(R, K * C)
    return {"aug": aug.astype(BF16), "bigc": bigc, "rows": rows,
            "uniq": uniq, "valid": valid.reshape(N)}


def _finish_prep(prep, ntu, slotmap):
    """Phase 2: place unique rows at slots, build item->slot maps."""
    uniq = prep["uniq"]
    upad = np.concatenate([uniq, np.full(ntu * 128 - len(uniq), uniq[0],
                                         np.int64)])
    rowidx = upad[slotmap].astype(np.int32)                   # (128, ntu)
    rank_of_item = np.searchsorted(uniq, prep["rows"])        # (N,)
    # inverse of slotmap: rank -> (p, t)
    inv_p = np.empty(ntu * 128, np.int64)
    inv_t = np.empty(ntu * 128, np.int64)
    pp, tt = np.indices(slotmap.shape)
    inv_p[slotmap.reshape(-1)] = pp.reshape(-1)
    inv_t[slotmap.reshape(-1)] = tt.reshape(-1)
    nd = min(NDENSE, ntu)
    gdense = prep["aug"][rowidx[:, :nd].reshape(-1)].reshape(128, -1)
    in_map = {"aug": prep["aug"], "bigc": prep["bigc"], "rowidx": rowidx,
              "gdense": gdense}
    return in_map, (inv_p[rank_of_item], inv_t[rank_of_item], prep["valid"])


def _make_consts():
    pmat1 = np.zeros((K * K, NP_), np.float32)
    for p in range(NP_):
        for k in range(K):
            pmat1[k * K + PERMS[p, k], p] = 1.0
    pmat = np.zeros((128, 8 * NP_), np.float32)
    for tblk in range(8):
        pmat[tblk * 16:(tblk + 1) * 16, tblk * NP_:(tblk + 1) * NP_] = pmat1
    return {"pmat": pmat.astype(BF16), "ident": np.eye(128, dtype=BF16)}


def _build(nc, bass, mybir, tile, ntu):
    AP = bass.AP
    dt = mybir.dt
    Alu = mybir.AluOpType
    Act = mybir.ActivationFunctionType

    nd = min(NDENSE, ntu)
    aug = nc.dram_tensor("aug", [R, K * C], dt.bfloat16, kind="ExternalInput")
    bigc = nc.dram_tensor("bigc", [128, W_BIGC], dt.bfloat16,
                          kind="ExternalInput")
    rowidx = nc.dram_tensor("rowidx", [128, ntu], dt.int32,
                            kind="ExternalInput")
    gdense = nc.dram_tensor("gdense", [128, nd * K * C], dt.bfloat16,
                            kind="ExternalInput")
    mint_o = nc.dram_tensor("mint_o", [128, NT], dt.float32,
                            kind="ExternalOutput")
    s16_o = nc.dram_tensor("s16_o", [128, ntu * K * 16], dt.bfloat16,
                           kind="ExternalOutput")

    with tile.TileContext(nc) as tc:
        with (
            tc.tile_pool(name="main", bufs=1) as pool,
            tc.tile_pool(name="psum", bufs=1, space="PSUM") as ppool,
        ):
            # rowidx first: it gates the indirect gather stream
            ri = pool.tile([128, ntu], dt.int32)
            nc.sync.dma_start(ri[:], rowidx.ap())
            # dense-primed leading tiles land on fast HW queues while the
            # SWDGE warms up; finely split so exp starts ASAP
            G = pool.tile([128, ntu * K * C], dt.bfloat16)
            for st, w in _dense_parts(ntu):
                nc.sync.dma_start(
                    G[:, st * 1024:(st + w) * 1024],
                    gdense.ap()[:, st * 1024:(st + w) * 1024])
            cb = pool.tile([128, W_BIGC], dt.bfloat16)
            nc.sync.dma_start(cb[:], bigc.ap())

            pmat_v = cb[:, O_PMAT:O_PMAT + 192]
            ident_v = cb[:, O_ID:O_ID + 128]

            # dummy 16-row gather warms the SW-DMA engines before rowidx
            # lands (offsets are memset, not DMA-dependent)
            dum_i = pool.tile([16, 1], dt.int32)
            nc.gpsimd.memset(dum_i[:], 0)
            dum_o = pool.tile([16, K * C], dt.bfloat16)
            nc.gpsimd.indirect_dma_start(
                out=dum_o[:], out_offset=None, in_=aug.ap(),
                in_offset=bass.IndirectOffsetOnAxis(ap=dum_i[:], axis=0))

            # ---- indirect row gathers, chunked for DMA/compute overlap
            for st, w in _chunks(ntu):
                nc.gpsimd.indirect_dma_start(
                    out=G[:, st * 1024:(st + w) * 1024], out_offset=None,
                    in_=aug.ap(),
                    in_offset=bass.IndirectOffsetOnAxis(ap=ri[:, st:st + w],
                                                        axis=0))

            # ---- 24-perm totals per half on PE, segmented min on DVE
            # (runs while the gathers stream; cost matrix is host-built)
            mint = pool.tile([128, NT], dt.float32)
            for h in range(2):
                pT = ppool.tile([128, 128], dt.bfloat16, tag=f"pT{h}")
                nc.tensor.transpose(out=pT[:],
                                    in_=cb[:, O_COST + h * 128:
                                           O_COST + (h + 1) * 128],
                                    identity=ident_v)
                cT = pool.tile([128, 128], dt.bfloat16, tag=f"cT{h}")
                nc.vector.tensor_copy(out=cT[:], in_=pT[:])
                ptot = ppool.tile([128, 8 * NP_], dt.float32, tag=f"ptot{h}")
                nc.tensor.matmul(out=ptot[:], lhsT=cT[:], rhs=pmat_v,
                                 start=True, stop=True)
                nc.vector.tensor_reduce(
                    out=mint[:, h * 8:(h + 1) * 8],
                    in_=ptot[:].rearrange("p (t q) -> p t q", q=NP_),
                    axis=mybir.AxisListType.X, op=Alu.min)
            nc.sync.dma_start(mint_o.ap(), mint[:])

            # ---- exp (bf16), pipelined behind the gather chunks
            E = pool.tile([128, ntu * K * C], dt.bfloat16)
            for st, w in _exps(ntu):
                nc.scalar.activation(out=E[:, st * 1024:(st + w) * 1024],
                                     in_=G[:, st * 1024:(st + w) * 1024],
                                     func=Act.Exp)

            # ---- softmax denominators: per group, a tree of 2x-mode bf16
            # adds collapses (segs x 256) -> (segs x 16); host sums the 16.
            s16 = pool.tile([128, ntu * K * 16], dt.bfloat16)
            tree = []
            for w in (128, 64, 32):
                trtile = pool.tile([128, 4 * K * w], dt.bfloat16,
                                   tag=f"tr{w}", name=f"tr{w}")
                tree.append(trtile)

            def ev(nseg, segstep, width, off):
                a = E[:, 0:1]
                return AP(a.tensor, a.offset + off,
                          [list(a.ap[0]), [segstep, nseg], [1, width]])

            def tv(tl, nseg, segstride, width, off):
                a = tl[:, 0:1]
                return AP(a.tensor, a.offset + off,
                          [list(a.ap[0]), [segstride, nseg], [1, width]])

            def sv(nseg, width, off):
                a = s16[:, 0:1]
                return AP(a.tensor, a.offset + off,
                          [list(a.ap[0]), [16, nseg], [1, width]])

            for gt, gw in _groups(ntu):
                ns = gw * K
                nc.vector.tensor_tensor(
                    out=tv(tree[0], ns, 128, 128, 0),
                    in0=ev(ns, 256, 128, gt * 1024),
                    in1=ev(ns, 256, 128, gt * 1024 + 128), op=Alu.add)
                nc.vector.tensor_tensor(
                    out=tv(tree[1], ns, 64, 64, 0),
                    in0=tv(tree[0], ns, 128, 64, 0),
                    in1=tv(tree[0], ns, 128, 64, 64), op=Alu.add)
                nc.vector.tensor_tensor(
                    out=tv(tree[2], ns, 32, 32, 0),
                    in0=tv(tree[1], ns, 64, 32, 0),
                    in1=tv(tree[1], ns, 64, 32, 32), op=Alu.add)
                nc.vector.tensor_tensor(
                    out=sv(ns, 16, gt * K * 16),
                    in0=tv(tree[2], ns, 32, 16, 0),
                    in1=tv(tree[2], ns, 32, 16, 16), op=Alu.add)
                nc.sync.dma_start(
                    s16_o.ap()[:, gt * K * 16:(gt + gw) * K * 16],
                    s16[:, gt * K * 16:(gt + gw) * K * 16])
    return nc


NCORES = 8
_COMPILED = {}


def _get_compiled(ntu):
    if ntu not in _COMPILED:
        import concourse.bacc as bacc
        import concourse.bass as bass
        import concourse.mybir as mybir
        import concourse.tile as tile
        nc = bacc.Bacc("TRN2", target_bir_lowering=False, debug=False,
                       num_devices=NCORES)
        _build(nc, bass, mybir, tile, ntu)
        nc.compile()
        _COMPILED[ntu] = nc
    return _COMPILED[ntu]


def kernel(time, amount, out_time, out_amount, out_cat_logits, cat, lengths,
           indices):
    from concourse.bass_utils import run_bass_kernel_spmd

    time = np.asarray(time, dtype=np.float32)
    amount = np.asarray(amount, dtype=np.float32)
    out_time = np.asarray(out_time, dtype=np.float32)
    out_amount = np.asarray(out_amount, dtype=np.float32)
    out_cat_logits = np.asarray(out_cat_logits, dtype=np.float32)
    cat = np.asarray(cat, dtype=np.int32)
    lengths = np.asarray(lengths, dtype=np.int32)
    indices = np.asarray(indices, dtype=np.int32)

    consts = _make_consts()
    preps = [
        _host_prep(c, time, amount, out_time, out_amount, out_cat_logits,
                   cat, lengths, indices, consts)
        for c in range(NCORES)
    ]
    ntu = max((len(p["uniq"]) + 127) // 128 for p in preps)
    slotmap = _slotmap(ntu)
    nc = _get_compiled(ntu)
    in_maps, maps = [], []
    for p in preps:
        m, mp = _finish_prep(p, ntu, slotmap)
        in_maps.append(m)
        maps.append(mp)
    res = run_bass_kernel_spmd(nc, in_maps, core_ids=list(range(NCORES)))
    ls, cn = 0.0, 0.0
    for c in range(NCORES):
        mint = res.results[c]["mint_o"].reshape(N)            # item n-major
        s16 = res.results[c]["s16_o"].astype(np.float32)
        s4 = s16.reshape(128, ntu * K, 16).sum(-1)            # (128, ntu*K)
        slse = np.log(s4).reshape(128, ntu, K).sum(-1)        # (128, ntu)
        ip, it, valid = maps[c]
        item = mint + slse[ip, it]
        ls += float((item * valid).sum())
        cn += float(valid.sum())
    return np.float32(ls / (cn * K))
